# revision 10
# baseline (speedup 1.0000x reference)
"""Trainium2 Bass kernel for nn_CotLayer (CoT attention layer).

Full computation per reference:
  kemb = relu(grouped_conv3x3(x, Wk, groups=4))
  w1   = relu(We1 @ [x; kemb])            (1x1)
  w2   = We2 @ w1 + be2                   (1x1) -> per-pixel 3x3 kernel weights
  xv   = Wv @ x                           (1x1)
  agg  = relu(sum_k shift_k(xv) * broadcast(w2_k))   (per-pixel local agg)
  gap  = mean_{H,W}(agg + kemb)           (global -> AllReduce across cores)
  attn = softmax-pair(SE-MLP(gap))
  out  = agg*attn0 + kemb*attn1

Sharding: 8 cores = (batch b in {0,1}) x (H-quarter q in {0..3}); each core
computes 64 output rows with a 1-row halo baked into its input slab.
"""

import numpy as np
import ml_dtypes
from contextlib import ExitStack

import concourse.bass as bass
import concourse.tile as tile
from concourse import bacc, mybir
from concourse.bass_utils import run_bass_kernel_spmd

F32 = mybir.dt.float32
F32R = mybir.dt.float32r
BF16 = mybir.dt.bfloat16
AL = mybir.AluOpType
AF = mybir.ActivationFunctionType

B, C, H, W = 2, 128, 256, 256
KSZ, SP = 3, 8
DIM = C
ATTN_CHS = 64
NCORES = 8
RQ = H // 4          # 64 rows per core
TR = 4               # output rows per macro-tile
NT = RQ // TR        # 16 macro-tiles per core
NPX = TR * W         # 1024 px per macro-tile


def _prep_weights(inputs):
    """Host-side weight reorganization (all numpy)."""
    Wk = np.asarray(inputs["Wk"], np.float32)      # [128, 32, 3, 3]
    We1 = np.asarray(inputs["We1"], np.float32)[:, :, 0, 0]   # [64, 256]
    We2 = np.asarray(inputs["We2"], np.float32)[:, :, 0, 0]   # [144, 64]
    be2 = np.asarray(inputs["be2"], np.float32)    # [144]
    Wv = np.asarray(inputs["Wv"], np.float32)[:, :, 0, 0]     # [128, 128]
    Ws1 = np.asarray(inputs["Ws1"], np.float32)[:, :, 0, 0]   # [64, 128]
    bs1 = np.asarray(inputs["bs1"], np.float32)    # [64]
    Ws2 = np.asarray(inputs["Ws2"], np.float32)[:, :, 0, 0]   # [256, 64]
    bs2 = np.asarray(inputs["bs2"], np.float32)    # [256]

    # Grouped 3x3 conv as 9 block-diagonal lhsT matrices [ic, tap, oc].
    wk = np.zeros((C, 9, C), np.float32)
    for t in range(9):
        a, b = divmod(t, 3)
        for g in range(4):
            blk = Wk[32 * g:32 * g + 32, :, a, b]       # [oc_local, ic_local]
            wk[32 * g:32 * g + 32, t, 32 * g:32 * g + 32] = blk.T
    w1x = np.ascontiguousarray(We1[:, :C].T)            # [128, 64]
    w1k = np.ascontiguousarray(We1[:, C:].T).astype(ml_dtypes.bfloat16)
    # We2 rows rearranged per tap with 8-fold group replication: [m, tap, c]
    cidx = (np.arange(C) // SP) * 9                      # [128]
    we2 = np.zeros((64, 9, C), np.float32)
    be2k = np.zeros((C, 9), np.float32)
    for t in range(9):
        we2[:, t, :] = We2[cidx + t, :].T
        be2k[:, t] = be2[cidx + t]
    we2 = we2.astype(ml_dtypes.bfloat16)
    wv = np.ascontiguousarray(Wv.T)                      # [128, 128]
    ws1 = np.ascontiguousarray(Ws1.T) / float(H * W)     # fold mean divisor
    ws2 = np.zeros((64, 2, C), np.float32)
    ws2[:, 0, :] = Ws2[0::2, :].T
    ws2[:, 1, :] = Ws2[1::2, :].T
    bs2r = np.zeros((C, 2), np.float32)
    bs2r[:, 0] = bs2[0::2]
    bs2r[:, 1] = bs2[1::2]
    return dict(
        wk=np.ascontiguousarray(wk),
        w1x=w1x,
        w1k=np.ascontiguousarray(w1k),
        we2=np.ascontiguousarray(we2),
        be2=np.ascontiguousarray(be2k),
        wv=wv,
        ws1=ws1,
        bs1=bs1.reshape(64, 1),
        ws2=np.ascontiguousarray(ws2),
        bs2=bs2r,
    )


# Engine assignment knobs (tune from profile).
WBAR_ACT_TAPS = set(range(9))      # taps whose psum->sbuf copy runs on ACT
XV_ACT = {"c": True, "l": True, "r": True}


def _build_kernel(nc):
    xs = nc.dram_tensor("xs", [C, RQ + 2, W + 2], F32R, kind="ExternalInput")
    wk_d = nc.dram_tensor("wk", [C, 9, C], F32R, kind="ExternalInput")
    w1x_d = nc.dram_tensor("w1x", [C, 64], F32R, kind="ExternalInput")
    w1k_d = nc.dram_tensor("w1k", [C, 64], BF16, kind="ExternalInput")
    we2_d = nc.dram_tensor("we2", [64, 9, C], BF16, kind="ExternalInput")
    be2_d = nc.dram_tensor("be2", [C, 9], F32, kind="ExternalInput")
    wv_d = nc.dram_tensor("wv", [C, C], F32R, kind="ExternalInput")
    ws1_d = nc.dram_tensor("ws1", [C, 64], F32, kind="ExternalInput")
    bs1_d = nc.dram_tensor("bs1", [64, 1], F32, kind="ExternalInput")
    ws2_d = nc.dram_tensor("ws2", [64, 2, C], F32, kind="ExternalInput")
    bs2_d = nc.dram_tensor("bs2", [C, 2], F32, kind="ExternalInput")
    out_d = nc.dram_tensor("out", [C, RQ * W], F32, kind="ExternalOutput")

    cc_in = nc.dram_tensor("cc_in", [C, 1], F32, kind="Internal")
    cc_out = nc.dram_tensor("cc_out", [C, 1], F32, kind="Internal")

    with tile.TileContext(nc) as tc, ExitStack() as ctx:
        singles = ctx.enter_context(tc.tile_pool(name="singles", bufs=1))
        xpool = ctx.enter_context(tc.tile_pool(name="xchunk", bufs=3))
        w1pool = ctx.enter_context(tc.tile_pool(name="w1p", bufs=2))
        wbpool = ctx.enter_context(tc.tile_pool(name="wbp", bufs=2))
        xvpool = ctx.enter_context(tc.tile_pool(name="xvp", bufs=2))
        prodp = ctx.enter_context(tc.tile_pool(name="prodp", bufs=3))
        accp = ctx.enter_context(tc.tile_pool(name="accp", bufs=2))
        p2pool = ctx.enter_context(tc.tile_pool(name="p2p", bufs=2))
        outp = ctx.enter_context(tc.tile_pool(name="outp", bufs=3))
        smallp = ctx.enter_context(tc.tile_pool(name="smallp", bufs=1))
        pbig = ctx.enter_context(tc.tile_pool(name="pbig", bufs=2, space="PSUM"))
        pwb = ctx.enter_context(tc.tile_pool(name="pwb", bufs=2, space="PSUM"))

        # --- weights into SBUF ---
        wk_sb = singles.tile([C, 9, C], F32R)
        nc.sync.dma_start(wk_sb, wk_d.ap())
        w1x_sb = singles.tile([C, 64], F32R)
        nc.sync.dma_start(w1x_sb, w1x_d.ap())
        w1k_sb = singles.tile([C, 64], BF16)
        nc.sync.dma_start(w1k_sb, w1k_d.ap())
        we2_sb = singles.tile([64, 9, C], BF16)
        nc.sync.dma_start(we2_sb, we2_d.ap())
        be2_sb = singles.tile([C, 9], F32)
        nc.sync.dma_start(be2_sb, be2_d.ap())
        wv_sb = singles.tile([C, C], F32R)
        nc.sync.dma_start(wv_sb, wv_d.ap())
        ws1_sb = singles.tile([C, 64], F32)
        nc.sync.dma_start(ws1_sb, ws1_d.ap())
        bs1_sb = singles.tile([64, 1], F32)
        nc.sync.dma_start(bs1_sb, bs1_d.ap())
        ws2_sb = singles.tile([64, 2, C], F32)
        nc.sync.dma_start(ws2_sb, ws2_d.ap())
        bs2_sb = singles.tile([C, 2], F32)
        nc.sync.dma_start(bs2_sb, bs2_d.ap())

        kemb_slab = singles.tile([C, RQ * W], BF16)
        agg_slab = singles.tile([C, RQ * W], BF16)
        slots_k = singles.tile([C, NT], F32)
        slots_a = singles.tile([C, NT], F32)
        attn_sb = singles.tile([C, 2], F32)

        # ---------------- phase 1 ----------------
        for t in range(NT):
            xc = xpool.tile([C, TR + 2, W + 2], F32R, tag="xc")
            nc.sync.dma_start(xc, xs.ap()[:, TR * t:TR * t + TR + 2, :])

            # key embedding: grouped 3x3 conv (block-diag), 9 taps x 2 row-pairs
            pk = pbig.tile([C, NPX], F32, tag="big")
            for tap in range(9):
                a, b = divmod(tap, 3)
                for g2 in range(2):
                    nc.tensor.matmul(
                        pk[:, g2 * 512:(g2 + 1) * 512],
                        lhsT=wk_sb[:, tap, :],
                        rhs=xc[:, 2 * g2 + a:2 * g2 + a + 2, b:b + W],
                        start=(tap == 0), stop=(tap == 8),
                    )
            kv = kemb_slab[:, t * NPX:(t + 1) * NPX]
            nc.vector.tensor_scalar(kv, pk, 0.0, None, AL.max, AL.add,
                                    accum_out=slots_k[:, t:t + 1])

            # w1 = relu(We1 @ [x; kemb])
            pw = pbig.tile([64, NPX], F32, tag="big")
            nc.tensor.matmul(pw[:, 0:512], lhsT=w1x_sb,
                             rhs=xc[:, 1:3, 1:1 + W], start=True, stop=False)
            nc.tensor.matmul(pw[:, 512:1024], lhsT=w1x_sb,
                             rhs=xc[:, 3:5, 1:1 + W], start=True, stop=False)
            nc.tensor.matmul(pw[:, 0:512], lhsT=w1k_sb, rhs=kv[:, 0:512],
                             start=False, stop=True)
            nc.tensor.matmul(pw[:, 512:1024], lhsT=w1k_sb, rhs=kv[:, 512:1024],
                             start=False, stop=True)
            w1b = w1pool.tile([64, NPX], BF16, tag="w1")
            nc.scalar.activation(w1b, pw, AF.Relu)

            # xv = Wv @ x over 6 rows; 3 column-shifted bf16 copies
            xvc = xvpool.tile([C, TR + 2, W], BF16, tag="xvc")
            xvl = xvpool.tile([C, TR + 2, W], BF16, tag="xvl")
            xvr = xvpool.tile([C, TR + 2, W], BF16, tag="xvr")
            for m in range(3):
                pxv = pwb.tile([C, 512], F32, tag="wb")
                nc.tensor.matmul(pxv, lhsT=wv_sb,
                                 rhs=xc[:, 2 * m:2 * m + 2, 1:1 + W],
                                 start=True, stop=True)
                pv = pxv.rearrange("p (r w) -> p r w", w=W)
                eng_c = nc.scalar if XV_ACT["c"] else nc.vector
                eng_l = nc.scalar if XV_ACT["l"] else nc.vector
                eng_r = nc.scalar if XV_ACT["r"] else nc.vector
                if eng_c is nc.scalar:
                    nc.scalar.activation(xvc[:, 2 * m:2 * m + 2, :], pv, AF.Copy)
                else:
                    nc.vector.tensor_scalar(xvc[:, 2 * m:2 * m + 2, :], pv,
                                            0.0, None, AL.add)
                if eng_l is nc.scalar:
                    nc.scalar.activation(xvl[:, 2 * m:2 * m + 2, 0:W - 1],
                                         pv[:, :, 1:W], AF.Copy)
                else:
                    nc.vector.tensor_scalar(xvl[:, 2 * m:2 * m + 2, 0:W - 1],
                                            pv[:, :, 1:W], 0.0, None, AL.add)
                if eng_r is nc.scalar:
                    nc.scalar.activation(xvr[:, 2 * m:2 * m + 2, 1:W],
                                         pv[:, :, 0:W - 1], AF.Copy)
                else:
                    nc.vector.tensor_scalar(xvr[:, 2 * m:2 * m + 2, 1:W],
                                            pv[:, :, 0:W - 1], 0.0, None, AL.add)
            nc.gpsimd.memset(xvl[:, :, W - 1:W], 0.0)
            nc.gpsimd.memset(xvr[:, :, 0:1], 0.0)

            # wbar_k = We2_k @ w1 + be2_k  (replicated per-pixel kernel weights)
            wbs = []
            for tap in range(9):
                pb = pwb.tile([C, NPX], F32, tag="wb")
                nc.tensor.matmul(pb[:, 0:512], lhsT=we2_sb[:, tap, :],
                                 rhs=w1b[:, 0:512], start=True, stop=True)
                nc.tensor.matmul(pb[:, 512:1024], lhsT=we2_sb[:, tap, :],
                                 rhs=w1b[:, 512:1024], start=True, stop=True)
                wb = wbpool.tile([C, NPX], BF16, tag=f"wb{tap}")
                if tap in WBAR_ACT_TAPS:
                    nc.scalar.activation(wb, pb, AF.Identity,
                                         bias=be2_sb[:, tap:tap + 1])
                else:
                    nc.vector.tensor_scalar(wb, pb, be2_sb[:, tap:tap + 1],
                                            None, AL.add)
                wbs.append(wb)

            # aggregation: agg = sum_k shift_k(xv) * wbar_k
            srcs = {0: xvr, 1: xvc, 2: xvl}
            acc = None
            for tap in range(9):
                a, b = divmod(tap, 3)
                xview = srcs[b][:, a:a + TR, :]
                if acc is None:
                    acc = accp.tile([C, NPX], BF16, tag="acc")
                    nc.vector.tensor_tensor(acc, xview, wbs[tap], AL.mult)
                else:
                    p = prodp.tile([C, NPX], BF16, tag="prod")
                    nc.vector.tensor_tensor(p, xview, wbs[tap], AL.mult)
                    acc2 = accp.tile([C, NPX], BF16, tag="acc")
                    nc.vector.tensor_tensor(acc2, acc, p, AL.add)
                    acc = acc2
            av = agg_slab[:, t * NPX:(t + 1) * NPX]
            nc.vector.tensor_scalar(av, acc, 0.0, None, AL.max, AL.add,
                                    accum_out=slots_a[:, t:t + 1])

        # ---------------- SE attention (tiny) ----------------
        sum_k = smallp.tile([C, 1], F32, tag="sk")
        sum_a = smallp.tile([C, 1], F32, tag="sa")
        nc.vector.tensor_reduce(sum_k, slots_k, mybir.AxisListType.X, AL.add)
        nc.vector.tensor_reduce(sum_a, slots_a, mybir.AxisListType.X, AL.add)
        gap = smallp.tile([C, 1], F32, tag="gap")
        nc.vector.tensor_tensor(gap, sum_k, sum_a, AL.add)
        nc.gpsimd.dma_start(cc_in.ap(), gap)
        nc.gpsimd.collective_compute(
            "AllReduce", AL.add,
            replica_groups=[[0, 1, 2, 3], [4, 5, 6, 7]],
            ins=[cc_in.ap().opt()],
            outs=[cc_out.ap().opt()],
        )
        gap2 = smallp.tile([C, 1], F32, tag="gap2")
        nc.gpsimd.dma_start(gap2, cc_out.ap())

        ph = pbig.tile([64, 1], F32, tag="big")
        nc.tensor.matmul(ph, lhsT=ws1_sb, rhs=gap2, start=True, stop=True)
        hso = smallp.tile([64, 1], F32, tag="h")
        nc.scalar.activation(hso, ph, AF.Relu, bias=bs1_sb[:, 0:1])
        pa = pbig.tile([C, 2], F32, tag="big")
        nc.tensor.matmul(pa[:, 0:1], lhsT=ws2_sb[:, 0, :], rhs=hso,
                         start=True, stop=True)
        nc.tensor.matmul(pa[:, 1:2], lhsT=ws2_sb[:, 1, :], rhs=hso,
                         start=True, stop=True)
        a01 = smallp.tile([C, 2], F32, tag="a01")
        nc.scalar.activation(a01[:, 0:1], pa[:, 0:1], AF.Identity,
                             bias=bs2_sb[:, 0:1])
        nc.scalar.activation(a01[:, 1:2], pa[:, 1:2], AF.Identity,
                             bias=bs2_sb[:, 1:2])
        dse = smallp.tile([C, 1], F32, tag="dse")
        nc.vector.tensor_tensor(dse, a01[:, 0:1], a01[:, 1:2], AL.subtract)
        nc.scalar.activation(attn_sb[:, 0:1], dse, AF.Sigmoid)
        nc.scalar.activation(attn_sb[:, 1:2], dse, AF.Sigmoid, scale=-1.0)

        # ---------------- phase 2: blend + store ----------------
        for t in range(NT):
            kv = kemb_slab[:, t * NPX:(t + 1) * NPX]
            av = agg_slab[:, t * NPX:(t + 1) * NPX]
            t1 = p2pool.tile([C, NPX], BF16, tag="t1")
            nc.vector.tensor_scalar(t1, kv, attn_sb[:, 1:2], None, AL.mult)
            outf = outp.tile([C, NPX], F32, tag="outf")
            nc.vector.scalar_tensor_tensor(outf, av, attn_sb[:, 0:1], t1,
                                           AL.mult, AL.add)
            nc.sync.dma_start(out_d.ap()[:, t * NPX:(t + 1) * NPX], outf)

    return nc


_CACHE = {}


def _get_nc():
    if "nc" not in _CACHE:
        nc = bacc.Bacc("TRN2", target_bir_lowering=False, debug=False,
                       num_devices=NCORES)
        _build_kernel(nc)
        nc.compile()
        _CACHE["nc"] = nc
    return _CACHE["nc"]


def kernel(**inputs):
    x = np.asarray(inputs["x"], np.float32)
    wts = _prep_weights(inputs)

    # Per-core x slabs with 1-px halo baked in: [128, 66, 258]
    xp = np.pad(x, ((0, 0), (0, 0), (1, 1), (1, 1)))
    in_maps = []
    for core in range(NCORES):
        bb, q = divmod(core, 4)
        slab = np.ascontiguousarray(xp[bb, :, RQ * q:RQ * q + RQ + 2, :])
        m = {"xs": slab}
        m.update(wts)
        in_maps.append(m)

    nc = _get_nc()
    res = run_bass_kernel_spmd(nc, in_maps, core_ids=list(range(NCORES)))
    out = np.empty((B, C, H, W), np.float32)
    for core in range(NCORES):
        bb, q = divmod(core, 4)
        out[bb, :, RQ * q:RQ * q + RQ, :] = \
            res.results[core]["out"].reshape(C, RQ, W)
    return out
